# revision 23
# baseline (speedup 1.0000x reference)
"""DomainBatchNorm Trainium2 kernel.

Math (per sample row r with one-hot domain mask m_r over D=8 domains):
    scale = gammas * rsqrt(pop_vars + eps)            # [D, F]
    shift = betas  - pop_means * scale                # [D, F]
    y[r]  = x[r] * (m_r @ scale) + (m_r @ shift)      # [B, F]

Strategy: pure data-parallel over the batch dim on 8 NeuronCores (4096 rows
per core).  Per 128-row tile, the [128, F] effective scale/shift are produced
on the TensorEngine as mask-tile @ table matmuls (K = D = 8).  The mask is
one-hot so it is exact in bf16; the fp32 scale/shift tables are fed through
the PE as a bf16 hi + bf16 lo split, accumulated in fp32 PSUM, which
reconstructs them to ~2^-18 relative accuracy.  The elementwise
y = x*es + et runs as two fp32 tensor_tensor ops on the VectorEngine.
The kernel is memory-roofline bound: 16 MiB in + 16 MiB out per core.
"""

import numpy as np
import ml_dtypes

B, F, D = 32768, 1024, 8
EPS = 1e-5
N_CORES = 8
ROWS = B // N_CORES          # 4096 rows per core
P = 128                      # partitions / rows per tile
N_TILES = ROWS // P          # 32
HALF = 512                   # fp32 matmul moving-operand max (one PSUM bank)

_NC_CACHE = {}


def _build_nc(reps=1, variant="full"):
    import concourse.bacc as bacc
    import concourse.tile as tile
    from concourse import mybir

    f32 = mybir.dt.float32
    bf16 = mybir.dt.bfloat16

    nc = bacc.Bacc(
        "TRN2", target_bir_lowering=False, debug=False, num_devices=N_CORES
    )

    x = nc.dram_tensor("x", [ROWS, F], f32, kind="ExternalInput").ap()
    maskT = nc.dram_tensor("maskT", [D, ROWS], bf16, kind="ExternalInput").ap()
    s_hi = nc.dram_tensor("s_hi", [D, F], bf16, kind="ExternalInput").ap()
    s_lo = nc.dram_tensor("s_lo", [D, F], bf16, kind="ExternalInput").ap()
    t_hi = nc.dram_tensor("t_hi", [D, F], bf16, kind="ExternalInput").ap()
    t_lo = nc.dram_tensor("t_lo", [D, F], bf16, kind="ExternalInput").ap()
    y = nc.dram_tensor("y", [ROWS, F], f32, kind="ExternalOutput").ap()

    # super-tile: SUP row-tiles of 128 rows move as ONE DMA (amortizes the
    # per-InstDMACopy fixed cost on the HWDGE ring); loads issue on the SP
    # ring, stores on the ACT ring so the two directions don't serialize on
    # one HWDGE FIFO.
    SUP = 1                      # row-tiles per super-tile
    store_eng = "scalar"
    BUFS = 8
    alt = False
    for part in variant.split("_"):
        if part.startswith("sup"):
            SUP = int(part[3:])
        if part in ("sp", "scalar", "gpsimd"):
            store_eng = part
        if part.startswith("b") and part[1:].isdigit():
            BUFS = int(part[1:])
        if part == "alt":
            alt = True
    N_SUP = N_TILES // SUP

    with tile.TileContext(nc) as tc:
        with (
            tc.tile_pool(name="consts", bufs=1) as consts,
            tc.tile_pool(name="xp", bufs=BUFS) as xp,
            tc.tile_pool(name="tmpp", bufs=4) as tmpp,
            tc.tile_pool(name="outp", bufs=BUFS) as outp,
            tc.tile_pool(name="psp", bufs=2, space="PSUM") as psp,
            tc.tile_pool(name="ptp", bufs=2, space="PSUM") as ptp,
        ):
            # consts go via the gpsimd (SWDGE) ring so they don't sit ahead
            # of the first x-tile loads in the SP HWDGE FIFO
            mT = consts.tile([D, ROWS], bf16)
            nc.gpsimd.dma_start(out=mT, in_=maskT)
            sh = consts.tile([D, F], bf16)
            nc.gpsimd.dma_start(out=sh, in_=s_hi)
            sl = consts.tile([D, F], bf16)
            nc.gpsimd.dma_start(out=sl, in_=s_lo)
            th = consts.tile([D, F], bf16)
            nc.gpsimd.dma_start(out=th, in_=t_hi)
            tl = consts.tile([D, F], bf16)
            nc.gpsimd.dma_start(out=tl, in_=t_lo)

            def body():
                for i in range(N_SUP):
                    r0 = i * SUP * P
                    load = nc.scalar if (alt and i % 2) else nc.sync
                    nc_store = nc.sync if (alt and i % 2) else None
                    if "storeonly" not in variant:
                        xt = xp.tile([P, SUP, F], f32)
                        load.dma_start(
                            out=xt,
                            in_=x[r0 : r0 + SUP * P, :].rearrange(
                                "(j p) f -> p j f", p=P
                            ),
                        )
                    if "loadonly" in variant:
                        continue
                    ot = outp.tile([P, SUP, F], f32)
                    if "storeonly" in variant:
                        nc.gpsimd.memset(ot, 0.0)
                    for j in range(SUP):
                        if "storeonly" in variant:
                            continue
                        if variant == "dma_copy":
                            nc.scalar.copy(ot[:, j, :], xt[:, j, :])
                            continue
                        w = mT[:, r0 + j * P : r0 + (j + 1) * P]  # [D, P] lhsT
                        ps = psp.tile([P, F], f32)  # eff_scale
                        pt = ptp.tile([P, F], f32)  # eff_shift
                        for h in (0, 1):
                            c = slice(h * HALF, (h + 1) * HALF)
                            nc.tensor.matmul(ps[:, c], lhsT=w, rhs=sh[:, c], start=True, stop=False)
                            nc.tensor.matmul(ps[:, c], lhsT=w, rhs=sl[:, c], start=False, stop=True)
                            nc.tensor.matmul(pt[:, c], lhsT=w, rhs=th[:, c], start=True, stop=False)
                            nc.tensor.matmul(pt[:, c], lhsT=w, rhs=tl[:, c], start=False, stop=True)

                        tmp = tmpp.tile([P, F], f32)
                        nc.vector.tensor_mul(tmp, xt[:, j, :], ps)
                        nc.vector.tensor_add(ot[:, j, :], tmp, pt)

                    if "loadonly" in variant:
                        continue
                    store = {"scalar": nc.scalar, "sp": nc.sync, "gpsimd": nc.gpsimd}[
                        store_eng
                    ]
                    if nc_store is not None:
                        store = nc_store
                    store.dma_start(
                        out=y[r0 : r0 + SUP * P, :].rearrange("(j p) f -> p j f", p=P),
                        in_=ot,
                    )

            if reps == 1:
                body()
            else:
                # bench mode: repeat the whole pipeline in a HW loop so one
                # NEFF execution carries `reps` kernel-equivalents of work
                with tc.For_i(0, reps, 1):
                    body()

    nc.compile()
    return nc


def _get_nc(reps=1, variant="full"):
    key = (reps, variant)
    if key not in _NC_CACHE:
        _NC_CACHE[key] = _build_nc(reps, variant)
    return _NC_CACHE[key]


def _prep_in_maps(inputs, mask, gammas, betas, pop_means, pop_vars):
    bf = ml_dtypes.bfloat16
    # Fold the per-domain params into scale/shift tables (tiny [D, F] work),
    # in float64 so the bf16 hi/lo split captures the true value.
    scale64 = gammas.astype(np.float64) / np.sqrt(pop_vars.astype(np.float64) + EPS)
    shift64 = betas.astype(np.float64) - pop_means.astype(np.float64) * scale64
    s_hi = scale64.astype(bf)
    s_lo = (scale64 - s_hi.astype(np.float64)).astype(bf)
    t_hi = shift64.astype(bf)
    t_lo = (shift64 - t_hi.astype(np.float64)).astype(bf)

    maskT = np.ascontiguousarray(mask.astype(bf).T)  # one-hot: exact in bf16

    in_maps = []
    for c in range(N_CORES):
        r0, r1 = c * ROWS, (c + 1) * ROWS
        in_maps.append(
            {
                "x": np.ascontiguousarray(inputs[r0:r1]),
                "maskT": np.ascontiguousarray(maskT[:, r0:r1]),
                "s_hi": s_hi,
                "s_lo": s_lo,
                "t_hi": t_hi,
                "t_lo": t_lo,
            }
        )
    return in_maps


def kernel(inputs, mask, gammas, betas, pop_means, pop_vars, _trace=False, **_tr_kw):
    from concourse.bass_utils import run_bass_kernel_spmd

    in_maps = _prep_in_maps(inputs, mask, gammas, betas, pop_means, pop_vars)
    nc = _get_nc()
    res = run_bass_kernel_spmd(
        nc, in_maps, list(range(N_CORES)), trace=_trace, **_tr_kw
    )
    out = np.concatenate([res.results[c]["y"] for c in range(N_CORES)], axis=0)
    if _trace:
        kernel.last_results = res
    return out


# revision 28
# speedup vs baseline: 1.0141x; 1.0141x over previous
"""DomainBatchNorm Trainium2 kernel.

Math (per sample row r with one-hot domain mask m_r over D=8 domains):
    scale = gammas * rsqrt(pop_vars + eps)            # [D, F]
    shift = betas  - pop_means * scale                # [D, F]
    y[r]  = x[r] * (m_r @ scale) + (m_r @ shift)      # [B, F]

Strategy: pure data-parallel over the batch dim on 8 NeuronCores (4096 rows
per core).  Per 128-row tile, the [128, F] effective scale/shift are produced
on the TensorEngine as mask-tile @ table matmuls (K = D = 8).  The mask is
one-hot so it is exact in bf16; the fp32 scale/shift tables are fed through
the PE as a bf16 hi + bf16 lo split, accumulated in fp32 PSUM, which
reconstructs them to ~2^-18 relative accuracy.  The elementwise
y = x*es + et runs as two fp32 tensor_tensor ops on the VectorEngine.
The kernel is memory-roofline bound: 16 MiB in + 16 MiB out per core.
"""

import numpy as np
import ml_dtypes

B, F, D = 32768, 1024, 8
EPS = 1e-5
N_CORES = 8
ROWS = B // N_CORES          # 4096 rows per core
P = 128                      # partitions / rows per tile
N_TILES = ROWS // P          # 32
HALF = 512                   # fp32 matmul moving-operand max (one PSUM bank)

_NC_CACHE = {}


def _build_nc(reps=1, variant="full"):
    import concourse.bacc as bacc
    import concourse.tile as tile
    from concourse import mybir

    f32 = mybir.dt.float32
    bf16 = mybir.dt.bfloat16

    nc = bacc.Bacc(
        "TRN2", target_bir_lowering=False, debug=False, num_devices=N_CORES
    )

    x = nc.dram_tensor("x", [ROWS, F], f32, kind="ExternalInput").ap()
    maskT = nc.dram_tensor("maskT", [D, ROWS], bf16, kind="ExternalInput").ap()
    # scale/shift tables as 3-term bf16 splits: s ~= s0 + s1 + s2 exactly to
    # below fp32 ulp; the PE accumulates the terms in fp32 PSUM
    s_terms = [
        nc.dram_tensor(f"s{k}", [D, F], bf16, kind="ExternalInput").ap()
        for k in range(3)
    ]
    t_terms = [
        nc.dram_tensor(f"t{k}", [D, F], bf16, kind="ExternalInput").ap()
        for k in range(3)
    ]
    y = nc.dram_tensor("y", [ROWS, F], f32, kind="ExternalOutput").ap()

    # super-tile: SUP row-tiles of 128 rows move as ONE DMA (amortizes the
    # per-InstDMACopy fixed cost on the HWDGE ring); loads issue on the SP
    # ring, stores on the ACT ring so the two directions don't serialize on
    # one HWDGE FIFO.
    SUP = 1                      # row-tiles per super-tile
    store_eng = "scalar"
    BUFS = 8
    alt = False
    NTERMS = 3                   # bf16 split terms for the tables
    for part in variant.split("_"):
        if part.startswith("sup"):
            SUP = int(part[3:])
        if part in ("sp", "scalar", "gpsimd"):
            store_eng = part
        if part.startswith("b") and part[1:].isdigit():
            BUFS = int(part[1:])
        if part == "alt":
            alt = True
        if part == "duo":
            NTERMS = 2
    N_SUP = N_TILES // SUP

    with tile.TileContext(nc) as tc:
        with (
            tc.tile_pool(name="consts", bufs=1) as consts,
            tc.tile_pool(name="xp", bufs=BUFS) as xp,
            tc.tile_pool(name="tmpp", bufs=4) as tmpp,
            tc.tile_pool(name="outp", bufs=BUFS) as outp,
            tc.tile_pool(name="psp", bufs=2, space="PSUM") as psp,
            tc.tile_pool(name="ptp", bufs=2, space="PSUM") as ptp,
        ):
            # consts go via the gpsimd (SWDGE) ring so they don't sit ahead
            # of the first x-tile loads in the SP HWDGE FIFO
            mT = consts.tile([D, ROWS], bf16)
            nc.gpsimd.dma_start(out=mT, in_=maskT)
            s_sb, t_sb = [], []
            for k in range(NTERMS):
                s_k = consts.tile([D, F], bf16, tag=f"s{k}")
                nc.gpsimd.dma_start(out=s_k, in_=s_terms[k])
                s_sb.append(s_k)
                t_k = consts.tile([D, F], bf16, tag=f"t{k}")
                nc.gpsimd.dma_start(out=t_k, in_=t_terms[k])
                t_sb.append(t_k)

            def body():
                for i in range(N_SUP):
                    r0 = i * SUP * P
                    load = nc.scalar if (alt and i % 2) else nc.sync
                    nc_store = nc.sync if (alt and i % 2) else None
                    if "storeonly" not in variant:
                        xt = xp.tile([P, SUP, F], f32)
                        load.dma_start(
                            out=xt,
                            in_=x[r0 : r0 + SUP * P, :].rearrange(
                                "(j p) f -> p j f", p=P
                            ),
                        )
                    if "loadonly" in variant:
                        continue
                    ot = outp.tile([P, SUP, F], f32)
                    if "storeonly" in variant:
                        nc.gpsimd.memset(ot, 0.0)
                    for j in range(SUP):
                        if "storeonly" in variant:
                            continue
                        if variant == "dma_copy":
                            nc.scalar.copy(ot[:, j, :], xt[:, j, :])
                            continue
                        w = mT[:, r0 + j * P : r0 + (j + 1) * P]  # [D, P] lhsT
                        ps = psp.tile([P, F], f32)  # eff_scale
                        pt = ptp.tile([P, F], f32)  # eff_shift
                        for h in (0, 1):
                            c = slice(h * HALF, (h + 1) * HALF)
                            for k in range(NTERMS):
                                nc.tensor.matmul(
                                    ps[:, c], lhsT=w, rhs=s_sb[k][:, c],
                                    start=(k == 0), stop=(k == NTERMS - 1),
                                )
                            for k in range(NTERMS):
                                nc.tensor.matmul(
                                    pt[:, c], lhsT=w, rhs=t_sb[k][:, c],
                                    start=(k == 0), stop=(k == NTERMS - 1),
                                )

                        tmp = tmpp.tile([P, F], f32)
                        nc.vector.tensor_mul(tmp, xt[:, j, :], ps)
                        nc.vector.tensor_add(ot[:, j, :], tmp, pt)

                    if "loadonly" in variant:
                        continue
                    store = {"scalar": nc.scalar, "sp": nc.sync, "gpsimd": nc.gpsimd}[
                        store_eng
                    ]
                    if nc_store is not None:
                        store = nc_store
                    store.dma_start(
                        out=y[r0 : r0 + SUP * P, :].rearrange("(j p) f -> p j f", p=P),
                        in_=ot,
                    )

            if reps == 1:
                body()
            else:
                # bench mode: repeat the whole pipeline in a HW loop so one
                # NEFF execution carries `reps` kernel-equivalents of work
                with tc.For_i(0, reps, 1):
                    body()

    nc.compile()
    return nc


def _get_nc(reps=1, variant="full"):
    key = (reps, variant)
    if key not in _NC_CACHE:
        _NC_CACHE[key] = _build_nc(reps, variant)
    return _NC_CACHE[key]


def _bf16_split(v64, nterms=3):
    """Split a float64 array into `nterms` bf16 arrays summing to ~v64."""
    bf = ml_dtypes.bfloat16
    terms, rem = [], v64
    for _ in range(nterms):
        t = rem.astype(bf)
        terms.append(t)
        rem = rem - t.astype(np.float64)
    return terms


def _prep_in_maps(inputs, mask, gammas, betas, pop_means, pop_vars):
    bf = ml_dtypes.bfloat16
    # Fold the per-domain params into scale/shift tables (tiny [D, F] work),
    # in float64 so the bf16 splits capture the true value below fp32 ulp.
    scale64 = gammas.astype(np.float64) / np.sqrt(pop_vars.astype(np.float64) + EPS)
    shift64 = betas.astype(np.float64) - pop_means.astype(np.float64) * scale64
    s_terms = _bf16_split(scale64)
    t_terms = _bf16_split(shift64)

    maskT = np.ascontiguousarray(mask.astype(bf).T)  # one-hot: exact in bf16

    in_maps = []
    for c in range(N_CORES):
        r0, r1 = c * ROWS, (c + 1) * ROWS
        im = {
            "x": np.ascontiguousarray(inputs[r0:r1]),
            "maskT": np.ascontiguousarray(maskT[:, r0:r1]),
        }
        for k in range(3):
            im[f"s{k}"] = s_terms[k]
            im[f"t{k}"] = t_terms[k]
        in_maps.append(im)
    return in_maps


def kernel(inputs, mask, gammas, betas, pop_means, pop_vars, _trace=False, **_tr_kw):
    from concourse.bass_utils import run_bass_kernel_spmd

    in_maps = _prep_in_maps(inputs, mask, gammas, betas, pop_means, pop_vars)
    nc = _get_nc()
    res = run_bass_kernel_spmd(
        nc, in_maps, list(range(N_CORES)), trace=_trace, **_tr_kw
    )
    out = np.concatenate([res.results[c]["y"] for c in range(N_CORES)], axis=0)
    if _trace:
        kernel.last_results = res
    return out
